# revision 31
# baseline (speedup 1.0000x reference)
"""CapsLayer2D Trainium2 kernel (8-core SPMD, data-parallel over batch).

Math per position p (of B*R*C) and capsule n, with U[i,o] = sum_e x[i,e]W[n,i,e,o]:
  s0 = (1/64) sum_i U_i ; v0 = squash(s0)
  s_{k+1} = s_k + sum_i (U_i . v_k) U_i ; v_{k+1} = squash(s_{k+1})
  out = v2   (the routing-b tensor never needs materializing)

Mapping:
  - 8 cores, 2 batches each -> 392 positions/core, 4 pos-blocks of 98.
  - Phase 1: s0 via dense K=1024 bf16 matmuls.
  - Phase 2 per block: u_hat to PSUM via block-diagonal-W bf16 matmuls; ACT
    drains PSUM into two bf16 layouts: ub (gi,n2,o) for the o-contraction and
    ubT (n2,o,gi) for the gi-contraction so both DVE products run in 2x
    (packed bf16) mode. Routing = products + halving add-trees on DVE.
  - Emission is software-pipelined: block b+1's matmuls/drains are emitted
    between block b's routing stages so ACT drains never queue ahead of the
    squash sqrts they would otherwise block.
"""
import numpy as np

import concourse.bacc as bacc
import concourse.bass as bass
import concourse.mybir as mybir
import concourse.tile as tile
from concourse.bass_utils import run_bass_kernel_spmd

N_CORES = 8
B, R, C = 16, 14, 14
N_IN, D_IN = 64, 16          # i, e
N_CAPS, CAPS_DIM = 10, 16    # n, o
IE = N_IN * D_IN             # 1024
POS = (B // N_CORES) * R * C # 392 positions per core
BLK = 98                     # pos-block size
NBLK = POS // BLK            # 4
NF = N_CAPS // 2             # 5 units of 2 capsules
NCH = IE // 128              # 8 contraction chunks
F32 = mybir.dt.float32
BF16 = mybir.dt.bfloat16
ACT_COPY = mybir.ActivationFunctionType.Copy


def build_kernel(repeat=1):
    nc = bacc.Bacc("TRN2", target_bir_lowering=False, debug=False,
                   num_devices=N_CORES)
    xTb = nc.dram_tensor("xTb", [128, NCH * POS], BF16,
                         kind="ExternalInput").ap()
    bdw = nc.dram_tensor("bdw", [128, NCH * N_CAPS * 128], BF16,
                         kind="ExternalInput").ap()
    wdb = nc.dram_tensor("wdb", [128, NCH * N_CAPS * 16], BF16,
                         kind="ExternalInput").ap()
    out = nc.dram_tensor("out", [POS, N_CAPS * 16], F32,
                         kind="ExternalOutput").ap()

    with tile.TileContext(nc) as tc:
        for _rep in range(repeat):
            with tc.tile_pool(name="const", bufs=1) as const, \
                 tc.tile_pool(name="work", bufs=3) as work:
                wdb_t = const.tile([128, NCH * N_CAPS * 16], BF16)
                nc.sync.dma_start(wdb_t[:], wdb[:])
                xtb_t = const.tile([128, NCH * POS], BF16)
                nc.sync.dma_start(xtb_t[:], xTb[:])
                bdw_t = const.tile([128, NF * 2048], BF16)   # f-major BD(W)
                for f in range(NF):
                    nc.sync.dma_start(bdw_t[:, f * 2048:(f + 1) * 2048],
                                      bdw[:, f * 2048:(f + 1) * 2048])
                s0_t = const.tile([BLK, NBLK * 160], F32)    # s0 per block
                v0_t = const.tile([BLK, NBLK * 160], BF16)   # v0 (P-mul rhs)
                out_t = const.tile([BLK, NBLK * 160], F32)

                def squash_head(s_ap):
                    """sq, reduce, sqrt (ACT), and the sqrt-independent DVE
                    ops. Returns (rt, rc) for squash_tail."""
                    s3 = s_ap.rearrange("p (n o) -> p n o", o=16)
                    sq = work.tile([BLK, 160], F32, tag="sq")
                    nc.vector.tensor_mul(
                        sq[:].rearrange("p (n o) -> p n o", o=16), s3, s3)
                    q = work.tile([BLK, N_CAPS], F32, tag="q")
                    nc.vector.tensor_reduce(
                        q[:], sq[:].rearrange("p (n o) -> p n o", o=16),
                        axis=mybir.AxisListType.X, op=mybir.AluOpType.add)
                    rt = work.tile([BLK, N_CAPS], F32, tag="rt")
                    nc.scalar.activation(rt[:], q[:],
                                         mybir.ActivationFunctionType.Sqrt)
                    qp = work.tile([BLK, N_CAPS], F32, tag="qp")
                    nc.vector.tensor_scalar_add(qp[:], q[:], 1.0)
                    rc = work.tile([BLK, N_CAPS], F32, tag="rc")
                    nc.vector.reciprocal(rc[:], qp[:])
                    return rt, rc

                def squash_tail(s_ap, v_ap, rt, rc):
                    al = work.tile([BLK, N_CAPS], F32, tag="al")
                    nc.vector.tensor_mul(al[:], rt[:], rc[:])
                    alb = al[:].unsqueeze(2).broadcast_to([BLK, N_CAPS, 16])
                    nc.vector.tensor_mul(
                        v_ap, s_ap.rearrange("p (n o) -> p n o", o=16), alb)

                # ---- phase 1: s0 = (1/64) sum_ie x*W (bf16 matmuls) ----
                with tc.tile_pool(name="psum_s", bufs=2, space="PSUM") as psum_s:
                    for b in range(NBLK):
                        ps = psum_s.tile([BLK, 160], F32, tag="ps")
                        for g in range(NCH):
                            nc.tensor.matmul(
                                ps[:],
                                xtb_t[:, g * POS + b * BLK:
                                      g * POS + (b + 1) * BLK],
                                wdb_t[:, g * 160:(g + 1) * 160],
                                start=(g == 0), stop=(g == NCH - 1))
                        nc.scalar.activation(s0_t[:, b * 160:(b + 1) * 160],
                                             ps[:], ACT_COPY, scale=1.0 / N_IN)

                # ---- phase 2 ----
                with tc.tile_pool(name="ubp", bufs=2) as ubp, \
                     tc.tile_pool(name="big", bufs=1) as big, \
                     tc.tile_pool(name="psum_u", bufs=2, space="PSUM") as psum_u:

                    ubs = [None] * NBLK   # (ub, ubT) tiles per block

                    def emit_mm(b):
                        ub = ubp.tile([BLK, NF * 2048], BF16, tag="ub")
                        ubT = ubp.tile([BLK, NF * 2048], BF16, tag="ubT")
                        ubs[b] = (ub, ubT)
                        ups = []
                        for f in range(NF):
                            up = psum_u.tile([BLK, 2048], F32, tag="up")
                            ups.append(up)
                            for g in range(NCH):
                                lhs = xtb_t[:, g * POS + b * BLK:
                                            g * POS + (b + 1) * BLK]
                                rhs = bdw_t[:, f * 2048 + g * 256:
                                            f * 2048 + (g + 1) * 256]
                                nc.tensor.matmul(
                                    up[:, g * 256:(g + 1) * 256], lhs, rhs,
                                    start=True, stop=True)
                        return ups

                    def emit_drain_ub(b, ups, f):
                        ub, _ = ubs[b]
                        nc.scalar.activation(ub[:, f * 2048:(f + 1) * 2048],
                                             ups[f][:], ACT_COPY)

                    def emit_drain_ubT(b, ups, f):
                        _, ubT = ubs[b]
                        up = ups[f]
                        upr = up[:].rearrange(
                            "p (g i8 n2 o) -> p n2 o g i8", g=8, i8=8, n2=2)
                        ubTr = ubT[:, f * 2048:(f + 1) * 2048].rearrange(
                            "p (n2 o g i8) -> p n2 o g i8", n2=2, o=16, g=8)
                        for n2 in range(2):
                            nc.scalar.activation(
                                ubTr[:, n2:n2 + 1].rearrange(
                                    "p n2 o g i8 -> p (n2 o) g i8"),
                                upr[:, n2:n2 + 1].rearrange(
                                    "p n2 o g i8 -> p (n2 o) g i8"),
                                ACT_COPY)

                    scur = [None] * NBLK

                    def rt_head(b, it, mid=None, groups=((0, NF),)):
                        """t-branch, q-branch, s-update, squash head (incl
                        the ACT sqrt). `mid` emits ACT work after the
                        Q-muls so it lands ahead of the sqrt in ACT's queue.
                        `groups` = contiguous f-ranges emitted separately so
                        a range's DVE chain can start as soon as its units'
                        drains land (used for the pipeline prologue).
                        Returns squash (rt, rc)."""
                        ub, ubT = ubs[b]
                        vb16 = v0_t[:, b * 160:(b + 1) * 160]
                        P = big.tile([BLK, NF * 2048], BF16, tag="prod")
                        t1 = big.tile([BLK, NF * 1024], BF16, tag="tr1")
                        t2 = big.tile([BLK, NF * 512], BF16, tag="tr2")
                        t3 = big.tile([BLK, NF * 256], BF16, tag="tr3")
                        agrT = work.tile([BLK, NF * 128], BF16, tag="agrT")
                        Q = big.tile([BLK, NF * 2048], BF16, tag="prod")
                        q1 = big.tile([BLK, NF * 1024], BF16, tag="tr1")
                        q2 = big.tile([BLK, NF * 512], BF16, tag="tr2")
                        q3 = big.tile([BLK, NF * 256], BF16, tag="tr3")
                        q4 = big.tile([BLK, NF * 128], BF16, tag="tr4")
                        q5 = big.tile([BLK, NF * 64], BF16, tag="tr5")
                        ds = work.tile([BLK, 160], F32, tag="ds")
                        for lo, hi in groups:
                            w = hi - lo
                            Ubg = ub[:, lo * 2048:hi * 2048].rearrange(
                                "p (f gi no) -> p f gi no", f=w, gi=64, no=32)
                            vbb = vb16[:, lo * 32:hi * 32].rearrange(
                                "p (f no) -> p f no", no=32) \
                                .unsqueeze(2).broadcast_to([BLK, w, 64, 32])
                            nc.vector.tensor_mul(
                                P[:, lo * 2048:hi * 2048].rearrange(
                                    "p (f gi no) -> p f gi no",
                                    f=w, gi=64, no=32), Ubg, vbb)
                            with nc.allow_low_precision("bf16 tree sums"):
                                Pv = P[:, lo * 2048:hi * 2048].rearrange(
                                    "p (s o) -> p s o", o=16)
                                t1v = t1[:, lo * 1024:hi * 1024].rearrange(
                                    "p (s o) -> p s o", o=8)
                                nc.vector.tensor_add(t1v, Pv[:, :, 0:8],
                                                     Pv[:, :, 8:16])
                                t2v = t2[:, lo * 512:hi * 512].rearrange(
                                    "p (s o) -> p s o", o=4)
                                nc.vector.tensor_add(t2v, t1v[:, :, 0:4],
                                                     t1v[:, :, 4:8])
                                t3v = t3[:, lo * 256:hi * 256].rearrange(
                                    "p (s o) -> p s o", o=2)
                                nc.vector.tensor_add(t3v, t2v[:, :, 0:2],
                                                     t2v[:, :, 2:4])
                                t3q = t3[:, lo * 256:hi * 256].rearrange(
                                    "p (f gi x) -> p f gi x", f=w, gi=64)
                                agrTo = agrT[:, lo * 128:hi * 128].rearrange(
                                    "p (f n2 gi) -> p f gi n2", f=w, n2=2)
                                nc.vector.tensor_add(agrTo,
                                                     t3q[:, :, :, 0:4:2],
                                                     t3q[:, :, :, 1:4:2])
                                agrb = agrT[:, lo * 128:hi * 128].rearrange(
                                    "p (fn gi) -> p fn gi", fn=2 * w) \
                                    .unsqueeze(2) \
                                    .broadcast_to([BLK, 2 * w, 16, 64])
                                UbTg = ubT[:, lo * 2048:hi * 2048].rearrange(
                                    "p (fn o gi) -> p fn o gi", fn=2 * w, o=16)
                                nc.vector.tensor_mul(
                                    Q[:, lo * 2048:hi * 2048].rearrange(
                                        "p (fn o gi) -> p fn o gi",
                                        fn=2 * w, o=16), UbTg, agrb)
                        if mid is not None:
                            mid()
                        with nc.allow_low_precision("bf16 tree sums"):
                            for lo, hi in groups:
                                Qv = Q[:, lo * 2048:hi * 2048].rearrange(
                                    "p (s g) -> p s g", g=64)
                                q1v = q1[:, lo * 1024:hi * 1024].rearrange(
                                    "p (s g) -> p s g", g=32)
                                nc.vector.tensor_add(q1v, Qv[:, :, 0:32],
                                                     Qv[:, :, 32:64])
                                q2v = q2[:, lo * 512:hi * 512].rearrange(
                                    "p (s g) -> p s g", g=16)
                                nc.vector.tensor_add(q2v, q1v[:, :, 0:16],
                                                     q1v[:, :, 16:32])
                                q3v = q3[:, lo * 256:hi * 256].rearrange(
                                    "p (s g) -> p s g", g=8)
                                nc.vector.tensor_add(q3v, q2v[:, :, 0:8],
                                                     q2v[:, :, 8:16])
                                q4v = q4[:, lo * 128:hi * 128].rearrange(
                                    "p (s g) -> p s g", g=4)
                                nc.vector.tensor_add(q4v, q3v[:, :, 0:4],
                                                     q3v[:, :, 4:8])
                                q5v = q5[:, lo * 64:hi * 64].rearrange(
                                    "p (s g) -> p s g", g=2)
                                nc.vector.tensor_add(q5v, q4v[:, :, 0:2],
                                                     q4v[:, :, 2:4])
                                nc.vector.tensor_add(
                                    ds[:, lo * 32:hi * 32].rearrange(
                                        "p (s g) -> p s g", g=1),
                                    q5v[:, :, 0:1], q5v[:, :, 1:2])
                        if it == 0:
                            s_cur = work.tile([BLK, 160], F32, tag="s_cur")
                            scur[b] = s_cur
                            nc.vector.tensor_add(
                                s_cur[:], s0_t[:, b * 160:(b + 1) * 160],
                                ds[:])
                        else:
                            s_cur = scur[b]
                            nc.vector.tensor_add(s_cur[:], s_cur[:], ds[:])
                        return squash_head(s_cur[:])

                    def rt_tail(b, it, rtrc):
                        rt, rc = rtrc
                        s_cur = scur[b]
                        if it == 0:
                            v_ap = v0_t[:, b * 160:(b + 1) * 160].rearrange(
                                "p (n o) -> p n o", o=16)
                        else:
                            v_ap = out_t[:, b * 160:(b + 1) * 160].rearrange(
                                "p (n o) -> p n o", o=16)
                        squash_tail(s_cur[:], v_ap, rt, rc)
                        if it == 1:
                            nc.sync.dma_start(
                                out[b * BLK:(b + 1) * BLK, :],
                                out_t[:, b * 160:(b + 1) * 160])

                    def emit_drain_unit(b, ups, f):
                        emit_drain_ub(b, ups, f)
                        emit_drain_ubT(b, ups, f)

                    # prologue: v0 squashes (ACT: sqrts ahead of drains),
                    # then block-0 matmuls + unit-order drains.
                    p1rt = [squash_head(s0_t[:, b * 160:(b + 1) * 160])
                            for b in range(NBLK)]
                    for b in range(NBLK):
                        squash_tail(s0_t[:, b * 160:(b + 1) * 160],
                                    v0_t[:, b * 160:(b + 1) * 160].rearrange(
                                        "p (n o) -> p n o", o=16), *p1rt[b])
                    ups0 = emit_mm(0)
                    # block-0 ub drains f0-f2 on DVE (idle during warmup) so
                    # ACT's drain queue halves and routing(0) starts sooner.
                    ub0, _ = ubs[0]
                    for f in range(3):
                        nc.vector.tensor_copy(ub0[:, f * 2048:(f + 1) * 2048],
                                              ups0[f][:])
                        emit_drain_ubT(0, ups0, f)
                    for f in (3, 4):
                        emit_drain_unit(0, ups0, f)

                    # pipelined main loop: next block's drains are spread
                    # across this block's routing (two units ahead of the
                    # first product, one per mid-routing slot) so the squash
                    # sqrts never queue behind a long drain burst on ACT.
                    for b in range(NBLK):
                        nb = b + 1
                        if nb < NBLK:
                            ups_next = emit_mm(nb)
                            emit_drain_unit(nb, ups_next, 0)
                            emit_drain_unit(nb, ups_next, 1)

                            def mid0(nb=nb, u=ups_next):
                                emit_drain_unit(nb, u, 2)

                            def slotB(nb=nb, u=ups_next):
                                emit_drain_unit(nb, u, 3)

                            def mid1(nb=nb, u=ups_next):
                                emit_drain_unit(nb, u, 4)
                        else:
                            mid0 = slotB = mid1 = None
                        groups = (tuple((f, f + 1) for f in range(NF))
                                  if b == 0 else ((0, NF),))
                        rtrc = rt_head(b, 0, mid=mid0, groups=groups)
                        rt_tail(b, 0, rtrc)
                        if slotB is not None:
                            slotB()
                        rtrc = rt_head(b, 1, mid=mid1)
                        rt_tail(b, 1, rtrc)

    nc.compile()
    return nc


def _host_prep(inputs, W):
    """Build per-core input maps from full inputs."""
    import ml_dtypes
    x = np.ascontiguousarray(inputs, dtype=np.float32).reshape(B, R * C, IE)
    Wf = np.ascontiguousarray(W, dtype=np.float32)  # [n, i, e, o]
    # bdw[(i8,e), (g,n,i8,o)]
    Wg = Wf.reshape(N_CAPS, 8, 8, D_IN, CAPS_DIM)   # [n, g, i8, e, o]
    bdw6 = np.zeros((8, D_IN, 8, 8, N_CAPS, CAPS_DIM), dtype=np.float32)
    for i8 in range(8):
        # [n, g, e, o] -> [e, g, n, o]
        bdw6[i8, :, :, i8, :, :] = Wg[:, :, i8, :, :].transpose(2, 1, 0, 3)
    # f-major: column addr = f*2048 + g*256 + i8*32 + n2*16 + o
    bdw = np.ascontiguousarray(
        bdw6.reshape(8, D_IN, 8, 8, NF, 2, CAPS_DIM)
        .transpose(0, 1, 4, 2, 3, 5, 6)
        .reshape(128, NCH * N_CAPS * 128)).astype(ml_dtypes.bfloat16)
    # wdb[(i8,e), (g, n, o)] dense
    wdb = np.ascontiguousarray(
        Wf.transpose(1, 2, 0, 3).reshape(NCH, 128, N_CAPS * 16)
        .transpose(1, 0, 2).reshape(128, NCH * N_CAPS * 16)
    ).astype(ml_dtypes.bfloat16)
    bpc = B // N_CORES
    in_maps = []
    for c in range(N_CORES):
        xc = x[c * bpc:(c + 1) * bpc].reshape(POS, IE)
        xt = np.ascontiguousarray(xc.T)                       # [1024, 392]
        xtb = np.ascontiguousarray(
            xt.reshape(NCH, 128, POS).transpose(1, 0, 2).reshape(128, NCH * POS)
        ).astype(ml_dtypes.bfloat16)
        in_maps.append({
            "xTb": xtb,
            "bdw": bdw,
            "wdb": wdb,
        })
    return in_maps


_NC_CACHE = []


def kernel(inputs: np.ndarray, W: np.ndarray) -> np.ndarray:
    in_maps = _host_prep(inputs, W)
    if not _NC_CACHE:
        _NC_CACHE.append(build_kernel())
    nc = _NC_CACHE[0]
    res = run_bass_kernel_spmd(nc, in_maps, list(range(N_CORES)))
    outs = [res.results[c]["out"] for c in range(N_CORES)]
    full = np.concatenate(outs, axis=0)  # [3136, 160]
    return full.reshape(B, R, C, N_CAPS, CAPS_DIM)


# revision 32
# speedup vs baseline: 1.2921x; 1.2921x over previous
"""CapsLayer2D Trainium2 kernel (8-core SPMD, data-parallel over batch).

Math per position p (of B*R*C) and capsule n, with U[i,o] = sum_e x[i,e]W[n,i,e,o]:
  s0 = (1/64) sum_i U_i ; v0 = squash(s0)
  s_{k+1} = s_k + sum_i (U_i . v_k) U_i ; v_{k+1} = squash(s_{k+1})
  out = v2   (the routing-b tensor never needs materializing)

Mapping:
  - 8 cores, 2 batches each -> 392 positions/core, 4 pos-blocks of 98.
  - Phase 1: s0 via dense K=1024 bf16 matmuls.
  - Phase 2 per block: u_hat to PSUM via block-diagonal-W bf16 matmuls; ACT
    drains PSUM into two bf16 layouts: ub (gi,n2,o) for the o-contraction and
    ubT (n2,o,gi) for the gi-contraction so both DVE products run in 2x
    (packed bf16) mode. Routing = products + halving add-trees on DVE.
  - Emission is software-pipelined: block b+1's matmuls/drains are emitted
    between block b's routing stages so ACT drains never queue ahead of the
    squash sqrts they would otherwise block.
"""
import numpy as np

import concourse.bacc as bacc
import concourse.bass as bass
import concourse.mybir as mybir
import concourse.tile as tile
from concourse.bass_utils import run_bass_kernel_spmd

N_CORES = 8
B, R, C = 16, 14, 14
N_IN, D_IN = 64, 16          # i, e
N_CAPS, CAPS_DIM = 10, 16    # n, o
IE = N_IN * D_IN             # 1024
POS = (B // N_CORES) * R * C # 392 positions per core
BLK = 98                     # pos-block size
NBLK = POS // BLK            # 4
NF = N_CAPS // 2             # 5 units of 2 capsules
NCH = IE // 128              # 8 contraction chunks
F32 = mybir.dt.float32
BF16 = mybir.dt.bfloat16
ACT_COPY = mybir.ActivationFunctionType.Copy


def build_kernel(repeat=1):
    nc = bacc.Bacc("TRN2", target_bir_lowering=False, debug=False,
                   num_devices=N_CORES)
    xTb = nc.dram_tensor("xTb", [128, NCH * POS], BF16,
                         kind="ExternalInput").ap()
    bdw = nc.dram_tensor("bdw", [128, NCH * N_CAPS * 128], BF16,
                         kind="ExternalInput").ap()
    wdb = nc.dram_tensor("wdb", [128, NCH * N_CAPS * 16], BF16,
                         kind="ExternalInput").ap()
    out = nc.dram_tensor("out", [POS, N_CAPS * 16], F32,
                         kind="ExternalOutput").ap()

    with tile.TileContext(nc) as tc:
        for _rep in range(repeat):
            with tc.tile_pool(name="const", bufs=1) as const, \
                 tc.tile_pool(name="work", bufs=3) as work:
                wdb_t = const.tile([128, NCH * N_CAPS * 16], BF16)
                nc.sync.dma_start(wdb_t[:], wdb[:])
                xtb_t = const.tile([128, NCH * POS], BF16)
                nc.sync.dma_start(xtb_t[:], xTb[:])
                bdw_t = const.tile([128, NF * 2048], BF16)   # f-major BD(W)
                for f in range(NF):
                    nc.sync.dma_start(bdw_t[:, f * 2048:(f + 1) * 2048],
                                      bdw[:, f * 2048:(f + 1) * 2048])
                s0_t = const.tile([BLK, NBLK * 160], F32)    # s0 per block
                v0_t = const.tile([BLK, NBLK * 160], BF16)   # v0 (P-mul rhs)
                out_t = const.tile([BLK, NBLK * 160], F32)

                def squash_head(s_ap):
                    """sq, reduce, sqrt (ACT), and the sqrt-independent DVE
                    ops. Returns (rt, rc) for squash_tail."""
                    s3 = s_ap.rearrange("p (n o) -> p n o", o=16)
                    sq = work.tile([BLK, 160], F32, tag="sq")
                    nc.vector.tensor_mul(
                        sq[:].rearrange("p (n o) -> p n o", o=16), s3, s3)
                    q = work.tile([BLK, N_CAPS], F32, tag="q")
                    nc.vector.tensor_reduce(
                        q[:], sq[:].rearrange("p (n o) -> p n o", o=16),
                        axis=mybir.AxisListType.X, op=mybir.AluOpType.add)
                    rt = work.tile([BLK, N_CAPS], F32, tag="rt")
                    nc.scalar.activation(rt[:], q[:],
                                         mybir.ActivationFunctionType.Sqrt)
                    qp = work.tile([BLK, N_CAPS], F32, tag="qp")
                    nc.vector.tensor_scalar_add(qp[:], q[:], 1.0)
                    rc = work.tile([BLK, N_CAPS], F32, tag="rc")
                    nc.vector.reciprocal(rc[:], qp[:])
                    return rt, rc

                def squash_tail(s_ap, v_ap, rt, rc):
                    al = work.tile([BLK, N_CAPS], F32, tag="al")
                    nc.vector.tensor_mul(al[:], rt[:], rc[:])
                    alb = al[:].unsqueeze(2).broadcast_to([BLK, N_CAPS, 16])
                    nc.vector.tensor_mul(
                        v_ap, s_ap.rearrange("p (n o) -> p n o", o=16), alb)

                # ---- phase 1: s0 = (1/64) sum_ie x*W (bf16 matmuls) ----
                with tc.tile_pool(name="psum_s", bufs=2, space="PSUM") as psum_s:
                    for b in range(NBLK):
                        ps = psum_s.tile([BLK, 160], F32, tag="ps")
                        for g in range(NCH):
                            nc.tensor.matmul(
                                ps[:],
                                xtb_t[:, g * POS + b * BLK:
                                      g * POS + (b + 1) * BLK],
                                wdb_t[:, g * 160:(g + 1) * 160],
                                start=(g == 0), stop=(g == NCH - 1))
                        nc.scalar.activation(s0_t[:, b * 160:(b + 1) * 160],
                                             ps[:], ACT_COPY, scale=1.0 / N_IN)

                # ---- phase 2 ----
                with tc.tile_pool(name="ubp", bufs=2) as ubp, \
                     tc.tile_pool(name="big", bufs=1) as big, \
                     tc.tile_pool(name="psum_u", bufs=2, space="PSUM") as psum_u:

                    ubs = [None] * NBLK   # (ub, ubT) tiles per block

                    def emit_mm(b):
                        ub = ubp.tile([BLK, NF * 2048], BF16, tag="ub")
                        ubT = ubp.tile([BLK, NF * 2048], BF16, tag="ubT")
                        ubs[b] = (ub, ubT)
                        ups = []
                        for f in range(NF):
                            up = psum_u.tile([BLK, 2048], F32, tag="up")
                            ups.append(up)
                            for g in range(NCH):
                                lhs = xtb_t[:, g * POS + b * BLK:
                                            g * POS + (b + 1) * BLK]
                                rhs = bdw_t[:, f * 2048 + g * 256:
                                            f * 2048 + (g + 1) * 256]
                                nc.tensor.matmul(
                                    up[:, g * 256:(g + 1) * 256], lhs, rhs,
                                    start=True, stop=True)
                        return ups

                    def emit_drain_ub(b, ups, f):
                        ub, _ = ubs[b]
                        nc.scalar.activation(ub[:, f * 2048:(f + 1) * 2048],
                                             ups[f][:], ACT_COPY)

                    def emit_drain_ubT(b, ups, f):
                        _, ubT = ubs[b]
                        up = ups[f]
                        upr = up[:].rearrange(
                            "p (g i8 n2 o) -> p n2 o g i8", g=8, i8=8, n2=2)
                        ubTr = ubT[:, f * 2048:(f + 1) * 2048].rearrange(
                            "p (n2 o g i8) -> p n2 o g i8", n2=2, o=16, g=8)
                        for n2 in range(2):
                            nc.scalar.activation(
                                ubTr[:, n2:n2 + 1].rearrange(
                                    "p n2 o g i8 -> p (n2 o) g i8"),
                                upr[:, n2:n2 + 1].rearrange(
                                    "p n2 o g i8 -> p (n2 o) g i8"),
                                ACT_COPY)

                    scur = [None] * NBLK

                    def rt_head(b, it, mid=None, groups=((0, NF),)):
                        """t-branch, q-branch, s-update, squash head (incl
                        the ACT sqrt). `mid` emits ACT work after the
                        Q-muls so it lands ahead of the sqrt in ACT's queue.
                        `groups` = contiguous f-ranges emitted separately so
                        a range's DVE chain can start as soon as its units'
                        drains land (used for the pipeline prologue).
                        Returns squash (rt, rc)."""
                        ub, ubT = ubs[b]
                        vb16 = v0_t[:, b * 160:(b + 1) * 160]
                        P = big.tile([BLK, NF * 2048], BF16, tag="prod")
                        t1 = big.tile([BLK, NF * 1024], BF16, tag="tr1")
                        t2 = big.tile([BLK, NF * 512], BF16, tag="tr2")
                        t3 = big.tile([BLK, NF * 256], BF16, tag="tr3")
                        agrT = work.tile([BLK, NF * 128], BF16, tag="agrT")
                        Q = big.tile([BLK, NF * 2048], BF16, tag="prod")
                        q1 = big.tile([BLK, NF * 1024], BF16, tag="tr1")
                        q2 = big.tile([BLK, NF * 512], BF16, tag="tr2")
                        q3 = big.tile([BLK, NF * 256], BF16, tag="tr3")
                        q4 = big.tile([BLK, NF * 128], BF16, tag="tr4")
                        q5 = big.tile([BLK, NF * 64], BF16, tag="tr5")
                        ds = work.tile([BLK, 160], BF16, tag="ds")
                        for lo, hi in groups:
                            w = hi - lo
                            Ubg = ub[:, lo * 2048:hi * 2048].rearrange(
                                "p (f gi no) -> p f gi no", f=w, gi=64, no=32)
                            vbb = vb16[:, lo * 32:hi * 32].rearrange(
                                "p (f no) -> p f no", no=32) \
                                .unsqueeze(2).broadcast_to([BLK, w, 64, 32])
                            nc.vector.tensor_mul(
                                P[:, lo * 2048:hi * 2048].rearrange(
                                    "p (f gi no) -> p f gi no",
                                    f=w, gi=64, no=32), Ubg, vbb)
                            with nc.allow_low_precision("bf16 tree sums"):
                                Pv = P[:, lo * 2048:hi * 2048].rearrange(
                                    "p (s o) -> p s o", o=16)
                                t1v = t1[:, lo * 1024:hi * 1024].rearrange(
                                    "p (s o) -> p s o", o=8)
                                nc.vector.tensor_add(t1v, Pv[:, :, 0:8],
                                                     Pv[:, :, 8:16])
                                t2v = t2[:, lo * 512:hi * 512].rearrange(
                                    "p (s o) -> p s o", o=4)
                                nc.vector.tensor_add(t2v, t1v[:, :, 0:4],
                                                     t1v[:, :, 4:8])
                                t3v = t3[:, lo * 256:hi * 256].rearrange(
                                    "p (s o) -> p s o", o=2)
                                nc.vector.tensor_add(t3v, t2v[:, :, 0:2],
                                                     t2v[:, :, 2:4])
                                t3q = t3[:, lo * 256:hi * 256].rearrange(
                                    "p (f gi x) -> p f gi x", f=w, gi=64)
                                agrTo = agrT[:, lo * 128:hi * 128].rearrange(
                                    "p (f n2 gi) -> p f gi n2", f=w, n2=2)
                                nc.vector.tensor_add(agrTo,
                                                     t3q[:, :, :, 0:4:2],
                                                     t3q[:, :, :, 1:4:2])
                                agrb = agrT[:, lo * 128:hi * 128].rearrange(
                                    "p (fn gi) -> p fn gi", fn=2 * w) \
                                    .unsqueeze(2) \
                                    .broadcast_to([BLK, 2 * w, 16, 64])
                                UbTg = ubT[:, lo * 2048:hi * 2048].rearrange(
                                    "p (fn o gi) -> p fn o gi", fn=2 * w, o=16)
                                nc.vector.tensor_mul(
                                    Q[:, lo * 2048:hi * 2048].rearrange(
                                        "p (fn o gi) -> p fn o gi",
                                        fn=2 * w, o=16), UbTg, agrb)
                        if mid is not None:
                            mid()
                        with nc.allow_low_precision("bf16 tree sums"):
                            for lo, hi in groups:
                                Qv = Q[:, lo * 2048:hi * 2048].rearrange(
                                    "p (s g) -> p s g", g=64)
                                q1v = q1[:, lo * 1024:hi * 1024].rearrange(
                                    "p (s g) -> p s g", g=32)
                                nc.vector.tensor_add(q1v, Qv[:, :, 0:32],
                                                     Qv[:, :, 32:64])
                                q2v = q2[:, lo * 512:hi * 512].rearrange(
                                    "p (s g) -> p s g", g=16)
                                nc.vector.tensor_add(q2v, q1v[:, :, 0:16],
                                                     q1v[:, :, 16:32])
                                q3v = q3[:, lo * 256:hi * 256].rearrange(
                                    "p (s g) -> p s g", g=8)
                                nc.vector.tensor_add(q3v, q2v[:, :, 0:8],
                                                     q2v[:, :, 8:16])
                                q4v = q4[:, lo * 128:hi * 128].rearrange(
                                    "p (s g) -> p s g", g=4)
                                nc.vector.tensor_add(q4v, q3v[:, :, 0:4],
                                                     q3v[:, :, 4:8])
                                q5v = q5[:, lo * 64:hi * 64].rearrange(
                                    "p (s g) -> p s g", g=2)
                                nc.vector.tensor_add(q5v, q4v[:, :, 0:2],
                                                     q4v[:, :, 2:4])
                                nc.vector.tensor_add(
                                    ds[:, lo * 32:hi * 32].rearrange(
                                        "p (s g) -> p s g", g=1),
                                    q5v[:, :, 0:1], q5v[:, :, 1:2])
                        if it == 0:
                            s_cur = work.tile([BLK, 160], F32, tag="s_cur")
                            scur[b] = s_cur
                            nc.vector.tensor_add(
                                s_cur[:], s0_t[:, b * 160:(b + 1) * 160],
                                ds[:])
                        else:
                            s_cur = scur[b]
                            nc.vector.tensor_add(s_cur[:], s_cur[:], ds[:])
                        return squash_head(s_cur[:])

                    def rt_tail(b, it, rtrc):
                        rt, rc = rtrc
                        s_cur = scur[b]
                        if it == 0:
                            v_ap = v0_t[:, b * 160:(b + 1) * 160].rearrange(
                                "p (n o) -> p n o", o=16)
                        else:
                            v_ap = out_t[:, b * 160:(b + 1) * 160].rearrange(
                                "p (n o) -> p n o", o=16)
                        squash_tail(s_cur[:], v_ap, rt, rc)
                        if it == 1:
                            nc.sync.dma_start(
                                out[b * BLK:(b + 1) * BLK, :],
                                out_t[:, b * 160:(b + 1) * 160])

                    def emit_drain_unit(b, ups, f):
                        emit_drain_ub(b, ups, f)
                        emit_drain_ubT(b, ups, f)

                    # prologue: v0 squashes (ACT: sqrts ahead of drains),
                    # then block-0 matmuls + unit-order drains.
                    p1rt = [squash_head(s0_t[:, b * 160:(b + 1) * 160])
                            for b in range(NBLK)]
                    for b in range(NBLK):
                        squash_tail(s0_t[:, b * 160:(b + 1) * 160],
                                    v0_t[:, b * 160:(b + 1) * 160].rearrange(
                                        "p (n o) -> p n o", o=16), *p1rt[b])
                    ups0 = emit_mm(0)
                    # block-0 ub drains f0-f2 on DVE (idle during warmup) so
                    # ACT's drain queue halves and routing(0) starts sooner.
                    ub0, _ = ubs[0]
                    for f in range(3):
                        nc.vector.tensor_copy(ub0[:, f * 2048:(f + 1) * 2048],
                                              ups0[f][:])
                        emit_drain_ubT(0, ups0, f)
                    for f in (3, 4):
                        emit_drain_unit(0, ups0, f)

                    # pipelined main loop: next block's drains are spread
                    # across this block's routing (two units ahead of the
                    # first product, one per mid-routing slot) so the squash
                    # sqrts never queue behind a long drain burst on ACT.
                    for b in range(NBLK):
                        nb = b + 1
                        if nb < NBLK:
                            ups_next = emit_mm(nb)
                            emit_drain_unit(nb, ups_next, 0)
                            emit_drain_unit(nb, ups_next, 1)

                            def mid0(nb=nb, u=ups_next):
                                emit_drain_unit(nb, u, 2)

                            def slotB(nb=nb, u=ups_next):
                                emit_drain_unit(nb, u, 3)

                            def mid1(nb=nb, u=ups_next):
                                emit_drain_unit(nb, u, 4)
                        else:
                            mid0 = slotB = mid1 = None
                        groups = (tuple((f, f + 1) for f in range(NF))
                                  if b == 0 else ((0, NF),))
                        rtrc = rt_head(b, 0, mid=mid0, groups=groups)
                        rt_tail(b, 0, rtrc)
                        if slotB is not None:
                            slotB()
                        rtrc = rt_head(b, 1, mid=mid1)
                        rt_tail(b, 1, rtrc)

    nc.compile()
    return nc


def _host_prep(inputs, W):
    """Build per-core input maps from full inputs."""
    import ml_dtypes
    x = np.ascontiguousarray(inputs, dtype=np.float32).reshape(B, R * C, IE)
    Wf = np.ascontiguousarray(W, dtype=np.float32)  # [n, i, e, o]
    # bdw[(i8,e), (g,n,i8,o)]
    Wg = Wf.reshape(N_CAPS, 8, 8, D_IN, CAPS_DIM)   # [n, g, i8, e, o]
    bdw6 = np.zeros((8, D_IN, 8, 8, N_CAPS, CAPS_DIM), dtype=np.float32)
    for i8 in range(8):
        # [n, g, e, o] -> [e, g, n, o]
        bdw6[i8, :, :, i8, :, :] = Wg[:, :, i8, :, :].transpose(2, 1, 0, 3)
    # f-major: column addr = f*2048 + g*256 + i8*32 + n2*16 + o
    bdw = np.ascontiguousarray(
        bdw6.reshape(8, D_IN, 8, 8, NF, 2, CAPS_DIM)
        .transpose(0, 1, 4, 2, 3, 5, 6)
        .reshape(128, NCH * N_CAPS * 128)).astype(ml_dtypes.bfloat16)
    # wdb[(i8,e), (g, n, o)] dense
    wdb = np.ascontiguousarray(
        Wf.transpose(1, 2, 0, 3).reshape(NCH, 128, N_CAPS * 16)
        .transpose(1, 0, 2).reshape(128, NCH * N_CAPS * 16)
    ).astype(ml_dtypes.bfloat16)
    bpc = B // N_CORES
    in_maps = []
    for c in range(N_CORES):
        xc = x[c * bpc:(c + 1) * bpc].reshape(POS, IE)
        xt = np.ascontiguousarray(xc.T)                       # [1024, 392]
        xtb = np.ascontiguousarray(
            xt.reshape(NCH, 128, POS).transpose(1, 0, 2).reshape(128, NCH * POS)
        ).astype(ml_dtypes.bfloat16)
        in_maps.append({
            "xTb": xtb,
            "bdw": bdw,
            "wdb": wdb,
        })
    return in_maps


_NC_CACHE = []


def kernel(inputs: np.ndarray, W: np.ndarray) -> np.ndarray:
    in_maps = _host_prep(inputs, W)
    if not _NC_CACHE:
        _NC_CACHE.append(build_kernel())
    nc = _NC_CACHE[0]
    res = run_bass_kernel_spmd(nc, in_maps, list(range(N_CORES)))
    outs = [res.results[c]["out"] for c in range(N_CORES)]
    full = np.concatenate(outs, axis=0)  # [3136, 160]
    return full.reshape(B, R, C, N_CAPS, CAPS_DIM)
